# revision 2
# baseline (speedup 1.0000x reference)
"""Trainium2 Bass kernel for nn_DepthAwareEPIBranch — v2 (fp8 DoubleRow).

Math (per core slab of 80 rows, C=128 on partitions, pixels on free dim):
  dh = masked 1x5 depthwise conv   (fp8 diag matmuls, cross-group batched)
  dv = masked 5x1 depthwise conv   (fp8 DoubleRow: consecutive row-shift
                                    taps paired, 2 matmuls-in-1)
  lh/lv = lrelu(dh/dv)             (ACT Prelu -> fp8; Pool fused max(0.1x,x))
  epi64 = 64*s*(A_h@lh + A_v@lv)   (ONE fp8 DoubleRow per group)
  epi8  = fp8 copy of epi64        (ACT)
  m1    = [w_dm1@epi8_g0 ; w_dm1@epi8_g1] stacked on partitions 0..63
          (ONE fp8 DoubleRow via block weights)
  lr1   = lrelu(m1)                (ACT Prelu -> bf16)
  m2    = (w_dm2/(64 s)) @ lr1     (bf16, per group, tile row offset 0/32)
  sgq   = (0.25*m2 + 0.5)/64       (DVE tensor_scalar)
  out   = x_bf16 + epi8*sgq        (DVE)

I/O: x streamed in as fp8(conv path) + bf16(residual path); out written bf16.
"""

import numpy as np

import concourse.bacc as bacc
import concourse.mybir as mybir
from concourse import tile
from concourse.bass_utils import run_bass_kernel_spmd

F32 = mybir.dt.float32
BF16 = mybir.dt.bfloat16
FP8 = mybir.dt.float8e4
AF = mybir.ActivationFunctionType
ALU = mybir.AluOpType
DR = mybir.MatmulPerfMode.DoubleRow

P = 128          # channels = partitions
A = 5            # angRes
W = 320          # image width
NB = W // A      # 64 angular blocks per row
RPC = 80         # rows per core (B*H / 8)
NG = RPC // A    # 16 angular row-groups per core
NPAIR = NG // 2  # 8 pairs
N_CORES = 8

ES = 64.0        # epi fp8 scale

# v-conv DoubleRow configs: (name, dA, dB) with weights A=diag(wv[:,dA+2]),
# B=diag(wv[:,dB+2]); None = zero weights.  Moving rows = (r+dm, r+dm+1).
VCFG = [
    ("v01", 0, 1),      # taps 0,+1        moving (r, r+1)
    ("vz2", None, 2),   # tap +2           moving (r+1, r+2)
    ("vm10", -1, 0),    # taps -1,0        moving (r-1, r)
    ("v12", 1, 2),      # taps +1,+2       moving (r+1, r+2)
    ("vm2m1", -2, -1),  # taps -2,-1       moving (r-2, r-1)
    ("vz0", None, 0),   # tap 0            moving (r-1, r)
]
VIDX = {name: i for i, (name, _, _) in enumerate(VCFG)}
# schedule per r: list of config names
VSCHED = {
    0: ["v01", "vz2"],
    1: ["vm10", "v12"],
    2: ["vm2m1", "v01", "vz2"],
    3: ["vm2m1", "v01"],
    4: ["vm2m1", "vz0"],
}


def _build_nc():
    nc = bacc.Bacc("TRN2", target_bir_lowering=False, debug=False)

    x8 = nc.dram_tensor("x8", [P, RPC, W], FP8, kind="ExternalInput")
    xb = nc.dram_tensor("xb", [P, RPC, W], BF16, kind="ExternalInput")
    whd = nc.dram_tensor("whd", [P, A, P], FP8, kind="ExternalInput")
    wvd = nc.dram_tensor("wvd", [P, len(VCFG), 2, P], FP8, kind="ExternalInput")
    aw = nc.dram_tensor("aw", [P, 2, P], FP8, kind="ExternalInput")
    w1t = nc.dram_tensor("w1t", [P, 2, 64], FP8, kind="ExternalInput")
    w2t = nc.dram_tensor("w2t", [64, P], BF16, kind="ExternalInput")
    ys = nc.dram_tensor("ys", [P, RPC, W], BF16, kind="ExternalOutput")

    with tile.TileContext(nc) as tc:
        with (
            tc.tile_pool(name="consts", bufs=1) as cp,
            tc.tile_pool(name="x8p", bufs=2) as x8p,
            tc.tile_pool(name="xbp", bufs=2) as xbp,
            tc.tile_pool(name="lhvp", bufs=2) as lhvp,
            tc.tile_pool(name="epi8p", bufs=2) as e8p,
            tc.tile_pool(name="lr1p", bufs=2) as lr1p,
            tc.tile_pool(name="sgqp", bufs=2) as sgp,
            tc.tile_pool(name="prodp", bufs=2) as prp,
            tc.tile_pool(name="outp", bufs=2) as op,
            tc.tile_pool(name="psdh", bufs=1, space="PSUM") as ppdh,
            tc.tile_pool(name="psdv", bufs=1, space="PSUM") as ppdv,
            tc.tile_pool(name="psepi", bufs=1, space="PSUM") as ppe,
            tc.tile_pool(name="psm1", bufs=1, space="PSUM") as ppm1,
            tc.tile_pool(name="psm2", bufs=1, space="PSUM") as ppm2,
        ):
            whd_t = cp.tile([P, A, P], FP8)
            nc.sync.dma_start(whd_t[:], whd[:])
            wvd_t = cp.tile([P, len(VCFG), 2, P], FP8)
            nc.sync.dma_start(wvd_t[:], wvd[:])
            aw_t = cp.tile([P, 2, P], FP8)
            nc.sync.dma_start(aw_t[:], aw[:])
            w1t_t = cp.tile([P, 2, 64], FP8)
            nc.sync.dma_start(w1t_t[:], w1t[:])
            w2t_t = cp.tile([64, P], BF16)
            nc.sync.dma_start(w2t_t[:], w2t[:])

            for pr in range(NPAIR):
                r0 = 2 * A * pr
                x8_t = x8p.tile([P, 2 * A, W], FP8, tag="x8")
                nc.sync.dma_start(x8_t[:], x8[:, r0 : r0 + 2 * A, :])
                xb_t = xbp.tile([P, 2 * A, W], BF16, tag="xb")
                nc.sync.dma_start(xb_t[:], xb[:, r0 : r0 + 2 * A, :])
                out_t = op.tile([P, 2 * A, W], BF16, tag="out")
                x8v = x8_t[:].rearrange("p r (b q) -> p r b q", q=A)

                for r in range(A):
                    # ---------- h-conv: masked diag taps, cross-group ----
                    dh_t = ppdh.tile([P, 2, 512], F32, tag="dh")
                    dv_t = ppdv.tile([P, 2, 512], F32, tag="dv")
                    dh = dh_t[:]
                    dv = dv_t[:]
                    dhb = dh[:, :, 0:W].rearrange("p g (b q) -> p g b q", q=A)
                    for k in range(A):
                        d = k - 2
                        j0 = max(0, -d)
                        L = A - abs(d)
                        for g in range(2):
                            nc.tensor.matmul(
                                dhb[:, g, :, j0 : j0 + L],
                                whd_t[:, k, :],
                                x8v[:, g * A + r, :, j0 + d : j0 + d + L],
                                start=(k == 0), stop=(k == A - 1),
                            )
                    # ---------- v-conv: fp8 DoubleRow tap pairs ----------
                    cfgs = VSCHED[r]
                    for g in range(2):
                        for i, name in enumerate(cfgs):
                            ci = VIDX[name]
                            _, dA, dB = VCFG[ci]
                            dm = dA if dA is not None else dB - 1
                            rm = g * A + r + dm
                            nc.tensor.matmul(
                                dv[:, g, 0:W],
                                wvd_t[:, ci],
                                x8_t[:, rm : rm + 2, :],
                                start=(i == 0), stop=(i == len(cfgs) - 1),
                                perf_mode=DR,
                            )
                    # ---------- leaky relu -> fp8 (ACT for h, Pool for v)
                    lhv = lhvp.tile([P, 2, 2, W], FP8, tag="lhv")
                    nc.scalar.activation(
                        lhv[:, :, 0, :], dh[:, :, 0:W], AF.Prelu, alpha=0.1
                    )
                    nc.scalar.activation(
                        lhv[:, :, 1, :], dv[:, :, 0:W], AF.Prelu, alpha=0.1
                    )
                    # ---------- epi64 = A_h@lh + A_v@lv (one DR per group)
                    epi = ppe.tile([P, 2, 512], F32, tag="epi")
                    for g in range(2):
                        nc.tensor.matmul(
                            epi[:, g, 0:W], aw_t[:], lhv[:, g],
                            start=True, stop=True, perf_mode=DR,
                        )
                    # ---------- epi -> fp8 SBUF ----------
                    epi8 = e8p.tile([P, 2, W], FP8, tag="epi8")
                    nc.scalar.copy(epi8[:], epi[:, :, 0:W])
                    # ---------- m1 stacked DoubleRow ----------
                    m1 = ppm1.tile([64, 512], F32, tag="m1")
                    nc.tensor.matmul(
                        m1[:, 0:W], w1t_t[:], epi8[:],
                        start=True, stop=True, perf_mode=DR,
                    )
                    lr1 = lr1p.tile([64, W], BF16, tag="lr1")
                    nc.scalar.activation(lr1[:], m1[:, 0:W], AF.Prelu, alpha=0.1)
                    # ---------- m2 per group (K=32 at part offset 32g) ---
                    sgq = sgp.tile([P, 2, W], BF16, tag="sgq")
                    for g in range(2):
                        m2 = ppm2.tile([P, 512], F32, tag="m2")
                        nc.tensor.matmul(
                            m2[:, 0:W],
                            w2t_t[32 * g : 32 * g + 32, :],
                            lr1[32 * g : 32 * g + 32, :],
                            start=True, stop=True,
                        )
                        # sgq = (0.25*m2+0.5)/64
                        nc.vector.tensor_scalar(
                            sgq[:, g, :], m2[:, 0:W], 0.25 / ES, 0.5 / ES,
                            ALU.mult, ALU.add,
                        )
                    # ---------- prod, out ----------
                    prod = prp.tile([P, 2, W], BF16, tag="prod")
                    nc.gpsimd.tensor_tensor(prod[:], epi8[:], sgq[:], ALU.mult)
                    nc.vector.tensor_tensor(
                        out_t[:].rearrange("p (g q) w -> p g q w", g=2)[:, :, r, :],
                        prod[:],
                        xb_t[:].rearrange("p (g q) w -> p g q w", g=2)[:, :, r, :],
                        ALU.add,
                    )

                nc.sync.dma_start(ys[:, r0 : r0 + 2 * A, :], out_t[:])

    nc.compile()
    return nc


def _prep_weights(w_h_dw, w_h_pw, w_v_dw, w_v_pw, w_dm1, w_dm2, w_fuse, scale):
    import ml_dtypes

    def f8(x):
        return np.ascontiguousarray(np.asarray(x, np.float32)).astype(
            ml_dtypes.float8_e4m3fn
        )

    def bf(x):
        return np.ascontiguousarray(np.asarray(x, np.float32)).astype(
            ml_dtypes.bfloat16
        )

    wh = np.asarray(w_h_dw, np.float32).reshape(P, A)
    wv = np.asarray(w_v_dw, np.float32).reshape(P, A)
    whp = np.asarray(w_h_pw, np.float32)[:, :, 0, 0]
    wvp = np.asarray(w_v_pw, np.float32)[:, :, 0, 0]
    w1 = np.asarray(w_dm1, np.float32)[:, :, 0, 0]
    w2 = np.asarray(w_dm2, np.float32)[:, :, 0, 0]
    wf = np.asarray(w_fuse, np.float32)[:, :, 0, 0]
    s = float(np.asarray(scale).reshape(-1)[0])

    a_h = ES * s * (wf[:, :P] @ whp)
    a_v = ES * s * (wf[:, P:] @ wvp)

    idx = np.arange(P)
    whd = np.zeros((P, A, P), np.float32)
    for k in range(A):
        whd[idx, k, idx] = wh[:, k]

    wvd = np.zeros((P, len(VCFG), 2, P), np.float32)
    for ci, (_, dA, dB) in enumerate(VCFG):
        if dA is not None:
            wvd[idx, ci, 0, idx] = wv[:, dA + 2]
        if dB is not None:
            wvd[idx, ci, 1, idx] = wv[:, dB + 2]

    w1t = np.zeros((P, 2, 64), np.float32)
    w1t[:, 0, 0:32] = w1.T
    w1t[:, 1, 32:64] = w1.T

    w2s = (w2 / (ES * s)).T          # [32, 128]
    w2t = np.zeros((64, P), np.float32)
    w2t[0:32] = w2s
    w2t[32:64] = w2s

    return {
        "whd": f8(whd),
        "wvd": f8(wvd),
        "aw": f8(np.stack([a_h.T, a_v.T], axis=1)),
        "w1t": f8(w1t),
        "w2t": bf(w2t),
    }


_NC_CACHE = None


def _get_nc():
    global _NC_CACHE
    if _NC_CACHE is None:
        _NC_CACHE = _build_nc()
    return _NC_CACHE


def _prep_inputs(x, wmap):
    import ml_dtypes

    x = np.asarray(x, np.float32)
    x8 = x.astype(ml_dtypes.float8_e4m3fn)
    xbf = x.astype(ml_dtypes.bfloat16)
    in_maps = []
    for k in range(N_CORES):
        b = k // 4
        r0 = (k % 4) * RPC
        m = {
            "x8": np.ascontiguousarray(x8[b, :, r0 : r0 + RPC, :]),
            "xb": np.ascontiguousarray(xbf[b, :, r0 : r0 + RPC, :]),
        }
        m.update(wmap)
        in_maps.append(m)
    return in_maps


def kernel(x, w_h_dw, w_h_pw, w_v_dw, w_v_pw, w_dm1, w_dm2, w_fuse, scale,
           angRes, **_unused):
    x = np.asarray(x, np.float32)
    B, C, H, Wd = x.shape
    assert (B, C, H, Wd) == (2, 128, 320, 320), x.shape
    assert int(np.asarray(angRes)) == A

    s = float(np.asarray(scale).reshape(-1)[0])
    if s == 0.0:
        return x.copy()

    wmap = _prep_weights(w_h_dw, w_h_pw, w_v_dw, w_v_pw, w_dm1, w_dm2,
                         w_fuse, scale)
    in_maps = _prep_inputs(x, wmap)

    nc = _get_nc()
    res = run_bass_kernel_spmd(nc, in_maps, list(range(N_CORES)))

    out = np.empty_like(x)
    for k in range(N_CORES):
        b = k // 4
        r0 = (k % 4) * RPC
        out[b, :, r0 : r0 + RPC, :] = np.asarray(
            res.results[k]["ys"], np.float32
        )
    return out
